# revision 4
# baseline (speedup 1.0000x reference)
"""Multi-head attention (B=4, S=2048, D=768, H=12) on 8 TRN2 NeuronCores.

Sharding: core = (batch b, query-half). Each core computes Q for its 1024
query rows and full-sequence K/V for its batch (K/V projection duplicated
across the 2 cores sharing a batch -> zero collectives), then SDPA + o_proj
for its rows. Output rows are disjoint across cores.

Device layout: "T-layout" [feature, seq] with features on partitions.
 - Q/K projected as QT/KT [768, S*] (bias fused via per-partition scalar add)
 - RoPE applied in T-layout (partition-shifted copy via DMA, sin/cos tables
   built on device from position_ids via rank-1 matmul + mod-2pi + Sin LUT)
 - scores computed TRANSPOSED: psum[sk, sq] = KT_h.T @ QT_h (two heads packed
   per matmul pair via tile_position row tiling, K=64 each)
 - exp fused into the psum->sbuf eviction on ScalarE (scale=1/8, no max-sub:
   scores are ~N(0,1) so exp overflow is impossible)
 - P@V directly consumes exp(scoresT) as the moving operand; V kept row-major
   [S, 768] with a ones column appended per head -> psum row 64 = softmax
   denominator for free
 - normalization deferred: attnT tiles scaled by broadcast 1/rowsum during
   psum eviction; o_proj emits row-major [sq, 768] f32.
"""

from contextlib import ExitStack

import numpy as np

import concourse.bass as bass
import concourse.bacc as bacc
import concourse.mybir as mybir
import concourse.tile as tile
from concourse.bass import ds, ts
from concourse.bass_utils import run_bass_kernel_spmd
from concourse.masks import make_identity

F32 = mybir.dt.float32
BF16 = mybir.dt.bfloat16
I32 = mybir.dt.int32
AF = mybir.ActivationFunctionType

B, S, D, H = 4, 2048, 768, 12
HD = 64
SQ = 1024          # query rows per core
DC = D // 128      # 6 d-chunks
ST = S // 128      # 16 seq tiles of 128
ROPE_BASE = 10000.0
TWO_PI = float(2.0 * np.pi)
N_CORES = 8


def build_nc():
    nc = bacc.Bacc("TRN2", target_bir_lowering=False, debug=False,
                   num_devices=N_CORES)

    hs = nc.dram_tensor("hs", [S, D], F32, kind="ExternalInput")
    hs_q = nc.dram_tensor("hs_q", [SQ, D], F32, kind="ExternalInput")
    pos = nc.dram_tensor("pos", [1, S], I32, kind="ExternalInput")
    pos_q = nc.dram_tensor("pos_q", [1, SQ], I32, kind="ExternalInput")
    wqT = nc.dram_tensor("wqT", [D, D], F32, kind="ExternalInput")
    wkT = nc.dram_tensor("wkT", [D, D], F32, kind="ExternalInput")
    wvT = nc.dram_tensor("wvT", [D, D], F32, kind="ExternalInput")
    woT = nc.dram_tensor("woT", [D, D], F32, kind="ExternalInput")
    bq = nc.dram_tensor("bq", [D, 1], F32, kind="ExternalInput")
    bk = nc.dram_tensor("bk", [D, 1], F32, kind="ExternalInput")
    bv = nc.dram_tensor("bv", [1, D], F32, kind="ExternalInput")
    out = nc.dram_tensor("out", [SQ, D], F32, kind="ExternalOutput")

    # turns-per-position for each of the 32 rope frequencies (constant)
    invf_turns_np = ((1.0 / ROPE_BASE) ** (np.arange(32) / 32.0) / TWO_PI
                     ).reshape(1, 32).astype(np.float32)
    invf_dram = nc.inline_tensor(invf_turns_np, name="invf_turns")

    with tile.TileContext(nc) as tc:
        _body(nc, tc, hs, hs_q, pos, pos_q, wqT, wkT, wvT, woT,
              bq, bk, bv, out, invf_dram)
    nc.compile()
    return nc


def _body(nc, tc, hs, hs_q, pos, pos_q, wqT, wkT, wvT, woT,
          bq, bk, bv, out, invf_dram):
  with ExitStack() as ctx:
    const = ctx.enter_context(tc.tile_pool(name="const", bufs=1))
    persist = ctx.enter_context(tc.tile_pool(name="persist", bufs=1))

    # ---- constants ----
    ident = const.tile([128, 128], F32, tag="ident")
    make_identity(nc, ident[:])
    ones_row = const.tile([1, 128], BF16, tag="ones_row")
    nc.gpsimd.memset(ones_row[:], 1.0)
    ones64 = const.tile([1, 64], BF16, tag="ones64")
    nc.gpsimd.memset(ones64[:], 1.0)
    invf = const.tile([1, 32], F32, tag="invf")
    nc.sync.dma_start(invf[:], invf_dram[:])

    # ---- rope tables (device-side, from position ids) ----
    def rope_tables(pos_dram, n, name):
        with (tc.tile_pool(name=f"rope_{name}", bufs=1) as rp,
              tc.tile_pool(name=f"rope_ps_{name}", bufs=1,
                           space="PSUM") as rps):
            posi = rp.tile([1, n], I32, tag="posi")
            nc.sync.dma_start(posi[:], pos_dram[:])
            posf = rp.tile([1, n], F32, tag="posf")
            nc.vector.tensor_copy(posf[:], posi[:])
            turns = rps.tile([32, n], F32, tag="turns")
            for i in range(n // 512):
                nc.tensor.matmul(turns[:, ts(i, 512)], invf[:],
                                 posf[:, ts(i, 512)], start=True, stop=True)
            sincos = {}
            for which, shift in (("sin", 0.0), ("cos", 0.25)):
                tsh = rp.tile([32, n], F32, tag="tsh")
                nc.vector.tensor_scalar_add(tsh[:], turns[:], shift)
                ti = rp.tile([32, n], I32, tag="ti")
                nc.vector.tensor_copy(ti[:], tsh[:])
                tif = rp.tile([32, n], F32, tag="tif")
                nc.vector.tensor_copy(tif[:], ti[:])
                frac = rp.tile([32, n], F32, tag="frac")
                nc.vector.tensor_sub(frac[:], tsh[:], tif[:])
                val = rp.tile([32, n], BF16, tag=f"val_{which}")
                nc.scalar.activation(val[:], frac[:], AF.Sin, scale=TWO_PI)
                sincos[which] = val
            sin32, cos32 = sincos["sin"], sincos["cos"]
            nsin32 = rp.tile([32, n], BF16, tag="nsin")
            nc.vector.tensor_scalar_mul(nsin32[:], sin32[:], -1.0)
            cosR = persist.tile([128, n], BF16, tag=f"cosR_{name}")
            sinS = persist.tile([128, n], BF16, tag=f"sinS_{name}")
            for q in range(4):
                nc.sync.dma_start(cosR[ds(32 * q, 32), :], cos32[:])
                src = nsin32 if q % 2 == 0 else sin32
                nc.sync.dma_start(sinS[ds(32 * q, 32), :], src[:])
        return cosR, sinS

    cosR_k, sinS_k = rope_tables(pos, S, "k")
    cosR_q, sinS_q = rope_tables(pos_q, SQ, "q")

    # persistent activation tensors
    QT = [persist.tile([128, SQ], BF16, tag=f"QT{e}", name=f"QT{e}")
          for e in range(DC)]
    KT = [persist.tile([128, S], BF16, tag=f"KT{e}", name=f"KT{e}")
          for e in range(DC)]
    Vaug = [persist.tile([128, H * 65], BF16, tag=f"Vaug{st}",
                         name=f"Vaug{st}") for st in range(ST)]
    attnT = [persist.tile([128, SQ], BF16, tag=f"attnT{e}", name=f"attnT{e}")
             for e in range(DC)]

    # load + cast one [768,768] weight into 6 bf16 chunks
    def load_weight(wT_dram, stage_pool, dst_pool, name):
        chunks = []
        for dc in range(DC):
            f = stage_pool.tile([128, D], F32, tag="wld", name="wld")
            nc.sync.dma_start(f[:], wT_dram[ts(dc, 128), :])
            c = dst_pool.tile([128, D], BF16, tag=f"w_{name}{dc}",
                              name=f"w_{name}{dc}")
            nc.vector.tensor_copy(c[:], f[:])
            chunks.append(c)
        return chunks

    # ---- projection super-stage ----
    with (tc.tile_pool(name="xt", bufs=1) as xt_pool,
          tc.tile_pool(name="hsload", bufs=5) as hl,
          tc.tile_pool(name="qkv_w", bufs=1) as qkv_w,
          tc.tile_pool(name="wstage", bufs=3) as wstage,
          tc.tile_pool(name="shift", bufs=2) as shp,
          tc.tile_pool(name="xpose_ps", bufs=2, space="PSUM") as xps,
          tc.tile_pool(name="proj_ps", bufs=3, space="PSUM") as pps):

        # transpose hidden states -> xT [768, S], xqT [768, SQ] (bf16)
        xT = [xt_pool.tile([128, S], BF16, tag=f"xT{dc}", name=f"xT{dc}")
              for dc in range(DC)]
        xqT = [xt_pool.tile([128, SQ], BF16, tag=f"xqT{dc}", name=f"xqT{dc}")
               for dc in range(DC)]

        def transpose_in(dst, src_dram, n_total):
            for sg in range(n_total // 512):
                tl = []
                for j in range(4):
                    t = hl.tile([128, D], F32, tag="hs_t", name="hs_t")
                    nc.sync.dma_start(t[:], src_dram[ts(sg * 4 + j, 128), :])
                    tl.append(t)
                for dc in range(DC):
                    p = xps.tile([128, 512], F32, tag="xp", name="xp")
                    for j in range(4):
                        nc.tensor.transpose(p[:, ts(j, 128)],
                                            tl[j][:, ts(dc, 128)], ident[:])
                    nc.vector.tensor_copy(dst[dc][:, ts(sg, 512)], p[:])

        transpose_in(xT, hs, S)
        transpose_in(xqT, hs_q, SQ)

        wq_sb = load_weight(wqT, wstage, qkv_w, "q")
        wk_sb = load_weight(wkT, wstage, qkv_w, "k")
        wv_sb = load_weight(wvT, wstage, qkv_w, "v")

        bq_sb = [qkv_w.tile([128, 1], F32, tag=f"bq{e}", name=f"bq{e}")
                 for e in range(DC)]
        bk_sb = [qkv_w.tile([128, 1], F32, tag=f"bk{e}", name=f"bk{e}")
                 for e in range(DC)]
        for e in range(DC):
            nc.sync.dma_start(bq_sb[e][:], bq[ts(e, 128), :])
            nc.sync.dma_start(bk_sb[e][:], bk[ts(e, 128), :])
        bv_f = qkv_w.tile([1, D], F32, tag="bv_f", name="bv_f")
        nc.sync.dma_start(bv_f[:], bv[:])
        bv_sb = qkv_w.tile([1, D], BF16, tag="bv", name="bv_sb")
        nc.vector.tensor_copy(bv_sb[:], bv_f[:])

        def proj_T(dst, w_sb, b_sb, x_chunks, n_total):
            for e in range(DC):
                for i in range(n_total // 512):
                    p = pps.tile([128, 512], F32, tag="proj", name="proj_p")
                    for dc in range(DC):
                        nc.tensor.matmul(p[:], w_sb[dc][:, ts(e, 128)],
                                         x_chunks[dc][:, ts(i, 512)],
                                         start=(dc == 0), stop=(dc == DC - 1))
                    nc.vector.tensor_scalar_add(dst[e][:, ts(i, 512)], p[:],
                                                b_sb[e][:])

        def rope_inplace(dst_chunks, cosR, sinS, n_total):
            for e in range(DC):
                sh = shp.tile([128, n_total], BF16, tag="shift", name="sh")
                for q in range(4):
                    src_q = (q // 2) * 2 + (1 - q % 2)  # 0<->32, 64<->96
                    nc.sync.dma_start(sh[ds(32 * q, 32), :],
                                      dst_chunks[e][ds(32 * src_q, 32), :])
                tmp = shp.tile([128, n_total], BF16, tag="ropetmp",
                               name="ropetmp")
                nc.vector.tensor_mul(tmp[:], sh[:], sinS[:])
                nc.vector.tensor_mul(dst_chunks[e][:], dst_chunks[e][:],
                                     cosR[:])
                nc.vector.tensor_add(dst_chunks[e][:], dst_chunks[e][:],
                                     tmp[:])

        proj_T(QT, wq_sb, bq_sb, xqT, SQ)
        rope_inplace(QT, cosR_q, sinS_q, SQ)
        proj_T(KT, wk_sb, bk_sb, xT, S)
        rope_inplace(KT, cosR_k, sinS_k, S)

        # V row-major with bias via ones-row rank-1 matmul
        for st in range(ST):
            for nt in range(2):
                p = pps.tile([128, 384], F32, tag="vproj", name="vproj_p")
                for dc in range(DC):
                    nc.tensor.matmul(p[:], xT[dc][:, ts(st, 128)],
                                     wv_sb[dc][:, ts(nt, 384)],
                                     start=(dc == 0), stop=False)
                nc.tensor.matmul(p[:], ones_row[:], bv_sb[:, ts(nt, 384)],
                                 start=False, stop=True)
                dst = Vaug[st].rearrange("p (h x) -> p h x", x=65)
                nc.vector.tensor_copy(
                    dst[:, ds(nt * 6, 6), 0:64],
                    p.rearrange("p (h hd) -> p h hd", hd=64))
            va = Vaug[st].rearrange("p (h x) -> p h x", x=65)
            nc.gpsimd.memset(va[:, :, 64:65], 1.0)

    # ---- attention + o_proj ----
    wop = ctx.enter_context(tc.tile_pool(name="wop", bufs=1))
    with tc.tile_pool(name="wostage", bufs=2) as wo_stage:
        wo_sb = load_weight(woT, wo_stage, wop, "o")

    with (tc.tile_pool(name="scores_ps", bufs=3, space="PSUM") as sps,
          tc.tile_pool(name="pv_ps", bufs=3, space="PSUM") as pvps,
          tc.tile_pool(name="rb_ps", bufs=2, space="PSUM") as rbps,
          tc.tile_pool(name="expp", bufs=8) as expp,
          tc.tile_pool(name="attn_sb", bufs=3) as asb):
        for hp in range(DC):          # head pair = e-chunk
            for sqt in range(SQ // 512):
                sq_sl = ts(sqt, 512)
                pv = [pvps.tile([65, 512], F32, tag="pv", name=f"pv{i}")
                      for i in range(2)]
                for skt in range(ST):
                    ex = []
                    for i in range(2):  # head within pair
                        sc = sps.tile([128, 512], F32, tag="sc", name="sc")
                        nc.tensor.matmul(
                            sc[:], KT[hp][ds(64 * i, 64), ts(skt, 128)],
                            QT[hp][ds(64 * i, 64), sq_sl],
                            start=True, stop=True,
                            tile_position=(64 * i, 0))
                        e = expp.tile([128, 512], BF16, tag="exp", name="expt")
                        nc.scalar.activation(e[:], sc[:], AF.Exp, scale=0.125)
                        ex.append(e)
                    for i in range(2):
                        h = 2 * hp + i
                        nc.tensor.matmul(
                            pv[i][:], Vaug[skt][:, ds(h * 65, 65)], ex[i][:],
                            start=(skt == 0), stop=(skt == ST - 1))
                for i in range(2):
                    rec = asb.tile([1, 512], F32, tag="rec", name="rec")
                    nc.vector.reciprocal(rec[:], pv[i][ds(64, 1), :])
                    recb = asb.tile([1, 512], BF16, tag="recb", name="recb")
                    nc.vector.tensor_copy(recb[:], rec[:])
                    rb = rbps.tile([64, 512], F32, tag="rb", name="rb")
                    nc.tensor.matmul(rb[:], ones64[:], recb[:],
                                     start=True, stop=True)
                    rbs = asb.tile([64, 512], BF16, tag="rbs", name="rbs")
                    nc.vector.tensor_copy(rbs[:], rb[:])
                    nc.vector.tensor_mul(attnT[hp][ds(64 * i, 64), sq_sl],
                                         pv[i][ds(0, 64), :], rbs[:])

    # ---- o_proj (row-major out) ----
    with (tc.tile_pool(name="o_ps", bufs=4, space="PSUM") as ops,
          tc.tile_pool(name="o_sb", bufs=4) as osb):
        for st in range(SQ // 128):
            for nt in range(2):
                p = ops.tile([128, 384], F32, tag="o", name="o_p")
                for dc in range(DC):
                    nc.tensor.matmul(p[:], attnT[dc][:, ts(st, 128)],
                                     wo_sb[dc][:, ts(nt, 384)],
                                     start=(dc == 0), stop=(dc == DC - 1))
                o = osb.tile([128, 384], F32, tag="o_out", name="o_out")
                nc.vector.tensor_copy(o[:], p[:])
                nc.sync.dma_start(out[ts(st, 128), ts(nt, 384)], o[:])


_NC_CACHE = None


def _get_nc():
    global _NC_CACHE
    if _NC_CACHE is None:
        _NC_CACHE = build_nc()
    return _NC_CACHE


def kernel(hidden_states, position_ids, wq, bq, wk, bk, wv, bv, wo,
           _trace=False):
    hidden_states = np.asarray(hidden_states, dtype=np.float32)
    position_ids = np.asarray(position_ids, dtype=np.int32)
    wqT = np.ascontiguousarray(np.asarray(wq, np.float32).T)
    wkT = np.ascontiguousarray(np.asarray(wk, np.float32).T)
    wvT = np.ascontiguousarray(np.asarray(wv, np.float32).T)
    woT = np.ascontiguousarray(np.asarray(wo, np.float32).T)
    bq_c = np.ascontiguousarray(np.asarray(bq, np.float32).reshape(D, 1))
    bk_c = np.ascontiguousarray(np.asarray(bk, np.float32).reshape(D, 1))
    bv_r = np.ascontiguousarray(np.asarray(bv, np.float32).reshape(1, D))

    nc = _get_nc()
    in_maps = []
    for core in range(N_CORES):
        b, half = core // 2, core % 2
        sl = slice(half * SQ, (half + 1) * SQ)
        in_maps.append({
            "hs": np.ascontiguousarray(hidden_states[b]),
            "hs_q": np.ascontiguousarray(hidden_states[b, sl]),
            "pos": np.ascontiguousarray(position_ids[b].reshape(1, S)),
            "pos_q": np.ascontiguousarray(position_ids[b, sl].reshape(1, SQ)),
            "wqT": wqT, "wkT": wkT, "wvT": wvT, "woT": woT,
            "bq": bq_c, "bk": bk_c, "bv": bv_r,
        })
    res = run_bass_kernel_spmd(nc, in_maps, list(range(N_CORES)),
                               trace=_trace)
    outp = np.empty((B, S, D), np.float32)
    for core in range(N_CORES):
        b, half = core // 2, core % 2
        outp[b, half * SQ:(half + 1) * SQ] = res.results[core]["out"]
    if _trace:
        kernel._last_exec_time_ns = res.exec_time_ns
        kernel._last_results = res
    return outp


# revision 11
# speedup vs baseline: 1.1505x; 1.1505x over previous
"""Multi-head attention (B=4, S=2048, D=768, H=12) on 8 TRN2 NeuronCores.

Sharding: core = (batch b, query-half). Each core computes Q for its 1024
query rows and full-sequence K/V for its batch (K/V projection duplicated
across the 2 cores sharing a batch -> zero collectives), then SDPA + o_proj
for its rows. Output rows are disjoint across cores.

Device layout: "T-layout" [feature, seq] with features on partitions.
 - Q/K projected as QT/KT [768, S*] (bias fused via per-partition scalar add)
 - RoPE applied in T-layout (partition-shifted copy via DMA, sin/cos tables
   built on device from position_ids via rank-1 matmul + mod-2pi + Sin LUT)
 - scores computed TRANSPOSED: psum[sk, sq] = KT_h.T @ QT_h (two heads packed
   per matmul pair via tile_position row tiling, K=64 each)
 - exp fused into the psum->sbuf eviction on ScalarE (scale=1/8, no max-sub:
   scores are ~N(0,1) so exp overflow is impossible)
 - P@V directly consumes exp(scoresT) as the moving operand; V kept row-major
   [S, 768] with a ones column appended per head -> psum row 64 = softmax
   denominator for free
 - normalization deferred: attnT tiles scaled by broadcast 1/rowsum during
   psum eviction; o_proj emits row-major [sq, 768] f32.
"""

from contextlib import ExitStack

import numpy as np

import concourse.bass as bass
import concourse.bacc as bacc
import concourse.mybir as mybir
import concourse.tile as tile
from concourse.bass import ds, ts
from concourse.bass_utils import run_bass_kernel_spmd
from concourse.masks import make_identity

F32 = mybir.dt.float32
BF16 = mybir.dt.bfloat16
I32 = mybir.dt.int32
AF = mybir.ActivationFunctionType

B, S, D, H = 4, 2048, 768, 12
HD = 64
SQ = 1024          # query rows per core
DC = D // 128      # 6 d-chunks
ST = S // 128      # 16 seq tiles of 128
ROPE_BASE = 10000.0
TWO_PI = float(2.0 * np.pi)
N_CORES = 8


def build_nc():
    nc = bacc.Bacc("TRN2", target_bir_lowering=False, debug=False,
                   num_devices=N_CORES)

    hs = nc.dram_tensor("hs", [S, D], F32, kind="ExternalInput")
    hs_q = nc.dram_tensor("hs_q", [SQ, D], F32, kind="ExternalInput")
    pos = nc.dram_tensor("pos", [1, S], I32, kind="ExternalInput")
    pos_q = nc.dram_tensor("pos_q", [1, SQ], I32, kind="ExternalInput")
    wqT = nc.dram_tensor("wqT", [D, D], F32, kind="ExternalInput")
    wkT = nc.dram_tensor("wkT", [D, D], F32, kind="ExternalInput")
    wvT = nc.dram_tensor("wvT", [D, D], F32, kind="ExternalInput")
    woT = nc.dram_tensor("woT", [D, D], F32, kind="ExternalInput")
    bq = nc.dram_tensor("bq", [D, 1], F32, kind="ExternalInput")
    bk = nc.dram_tensor("bk", [D, 1], F32, kind="ExternalInput")
    bv = nc.dram_tensor("bv", [1, D], F32, kind="ExternalInput")
    out = nc.dram_tensor("out", [SQ, D], F32, kind="ExternalOutput")

    # turns-per-position for each of the 32 rope frequencies (constant)
    invf_turns_np = ((1.0 / ROPE_BASE) ** (np.arange(32) / 32.0) / TWO_PI
                     ).reshape(1, 32).astype(np.float32)
    invf_dram = nc.inline_tensor(invf_turns_np, name="invf_turns")

    with tile.TileContext(nc) as tc:
        _body(nc, tc, hs, hs_q, pos, pos_q, wqT, wkT, wvT, woT,
              bq, bk, bv, out, invf_dram)
    nc.compile()
    return nc


def _body(nc, tc, hs, hs_q, pos, pos_q, wqT, wkT, wvT, woT,
          bq, bk, bv, out, invf_dram):
  with ExitStack() as ctx:
    const = ctx.enter_context(tc.tile_pool(name="const", bufs=1))
    persist = ctx.enter_context(tc.tile_pool(name="persist", bufs=1))

    # ---- constants ----
    ident = const.tile([128, 128], F32, tag="ident")
    make_identity(nc, ident[:])
    ones_row = const.tile([1, 128], BF16, tag="ones_row")
    nc.gpsimd.memset(ones_row[:], 1.0)
    ones64f = const.tile([1, 64], F32, tag="ones64f")
    nc.gpsimd.memset(ones64f[:], 1.0)
    invf = const.tile([1, 32], F32, tag="invf")
    nc.sync.dma_start(invf[:], invf_dram[:])

    # ---- rope tables (device-side, from position ids) ----
    def rope_tables(pos_dram, n, name):
        with (tc.tile_pool(name=f"rope_{name}", bufs=1) as rp,
              tc.tile_pool(name=f"rope_ps_{name}", bufs=1,
                           space="PSUM") as rps):
            posi = rp.tile([1, n], I32, tag="posi")
            nc.sync.dma_start(posi[:], pos_dram[:])
            posf = rp.tile([1, n], F32, tag="posf")
            nc.vector.tensor_copy(posf[:], posi[:])
            turns = rps.tile([32, n], F32, tag="turns")
            for i in range(n // 512):
                nc.tensor.matmul(turns[:, ts(i, 512)], invf[:],
                                 posf[:, ts(i, 512)], start=True, stop=True)
            sincos = {}
            for which, shift in (("sin", 0.0), ("cos", 0.25)):
                tsh = rp.tile([32, n], F32, tag="tsh")
                nc.vector.tensor_scalar_add(tsh[:], turns[:], shift)
                ti = rp.tile([32, n], I32, tag="ti")
                nc.vector.tensor_copy(ti[:], tsh[:])
                tif = rp.tile([32, n], F32, tag="tif")
                nc.vector.tensor_copy(tif[:], ti[:])
                frac = rp.tile([32, n], F32, tag="frac")
                nc.vector.tensor_sub(frac[:], tsh[:], tif[:])
                val = rp.tile([32, n], BF16, tag=f"val_{which}")
                nc.scalar.activation(val[:], frac[:], AF.Sin, scale=TWO_PI)
                sincos[which] = val
            sin32, cos32 = sincos["sin"], sincos["cos"]
            nsin32 = rp.tile([32, n], BF16, tag="nsin")
            nc.vector.tensor_scalar_mul(nsin32[:], sin32[:], -1.0)
            cosR = persist.tile([128, n], BF16, tag=f"cosR_{name}")
            sinS = persist.tile([128, n], BF16, tag=f"sinS_{name}")
            for q in range(4):
                nc.sync.dma_start(cosR[ds(32 * q, 32), :], cos32[:])
                src = nsin32 if q % 2 == 0 else sin32
                nc.sync.dma_start(sinS[ds(32 * q, 32), :], src[:])
        return cosR, sinS

    cosR_k, sinS_k = rope_tables(pos, S, "k")
    cosR_q, sinS_q = rope_tables(pos_q, SQ, "q")

    # persistent activation tensors
    QT = [persist.tile([128, SQ], BF16, tag=f"QT{e}", name=f"QT{e}")
          for e in range(DC)]
    KT = [persist.tile([128, S], BF16, tag=f"KT{e}", name=f"KT{e}")
          for e in range(DC)]
    Vaug = [persist.tile([128, H * 65], BF16, tag=f"Vaug{st}",
                         name=f"Vaug{st}") for st in range(ST)]
    attnT = [persist.tile([128, SQ], BF16, tag=f"attnT{e}", name=f"attnT{e}")
             for e in range(DC)]

    # load + cast one [768,768] weight into 6 bf16 chunks
    def load_weight(wT_dram, stage_pool, dst_pool, name):
        chunks = []
        for dc in range(DC):
            f = stage_pool.tile([128, D], F32, tag="wld", name="wld")
            nc.sync.dma_start(f[:], wT_dram[ts(dc, 128), :])
            c = dst_pool.tile([128, D], BF16, tag=f"w_{name}{dc}",
                              name=f"w_{name}{dc}")
            nc.scalar.copy(c[:], f[:])
            chunks.append(c)
        return chunks

    # ---- projection super-stage ----
    with (tc.tile_pool(name="xt", bufs=1) as xt_pool,
          tc.tile_pool(name="hsload", bufs=5) as hl,
          tc.tile_pool(name="qkv_w", bufs=1) as qkv_w,
          tc.tile_pool(name="wstage", bufs=3) as wstage,
          tc.tile_pool(name="shift", bufs=2) as shp,
          tc.tile_pool(name="xpose_ps", bufs=2, space="PSUM") as xps,
          tc.tile_pool(name="proj_ps", bufs=3, space="PSUM") as pps):

        # transpose hidden states -> xT [768, S], xqT [768, SQ] (bf16)
        xT = [xt_pool.tile([128, S], BF16, tag=f"xT{dc}", name=f"xT{dc}")
              for dc in range(DC)]
        xqT = [xt_pool.tile([128, SQ], BF16, tag=f"xqT{dc}", name=f"xqT{dc}")
               for dc in range(DC)]

        def transpose_in(dst, src_dram, n_total):
            for sg in range(n_total // 512):
                tl = []
                for j in range(4):
                    t = hl.tile([128, D], F32, tag="hs_t", name="hs_t")
                    nc.sync.dma_start(t[:], src_dram[ts(sg * 4 + j, 128), :])
                    tl.append(t)
                for dc in range(DC):
                    p = xps.tile([128, 512], F32, tag="xp", name="xp")
                    for j in range(4):
                        nc.tensor.transpose(p[:, ts(j, 128)],
                                            tl[j][:, ts(dc, 128)], ident[:])
                    nc.scalar.copy(dst[dc][:, ts(sg, 512)], p[:])

        transpose_in(xT, hs, S)
        transpose_in(xqT, hs_q, SQ)

        wq_sb = load_weight(wqT, wstage, qkv_w, "q")
        wk_sb = load_weight(wkT, wstage, qkv_w, "k")
        wv_sb = load_weight(wvT, wstage, qkv_w, "v")

        bq_sb = [qkv_w.tile([128, 1], F32, tag=f"bq{e}", name=f"bq{e}")
                 for e in range(DC)]
        bk_sb = [qkv_w.tile([128, 1], F32, tag=f"bk{e}", name=f"bk{e}")
                 for e in range(DC)]
        for e in range(DC):
            nc.sync.dma_start(bq_sb[e][:], bq[ts(e, 128), :])
            nc.sync.dma_start(bk_sb[e][:], bk[ts(e, 128), :])
        bv_f = qkv_w.tile([1, D], F32, tag="bv_f", name="bv_f")
        nc.sync.dma_start(bv_f[:], bv[:])
        bv_sb = qkv_w.tile([1, D], BF16, tag="bv", name="bv_sb")
        nc.vector.tensor_copy(bv_sb[:], bv_f[:])

        def proj_T(dst, w_sb, b_sb, x_chunks, n_total):
            for e in range(DC):
                for i in range(n_total // 512):
                    p = pps.tile([128, 512], F32, tag="proj", name="proj_p")
                    for dc in range(DC):
                        nc.tensor.matmul(p[:], w_sb[dc][:, ts(e, 128)],
                                         x_chunks[dc][:, ts(i, 512)],
                                         start=(dc == 0), stop=(dc == DC - 1))
                    nc.scalar.activation(dst[e][:, ts(i, 512)], p[:],
                                         AF.Identity, bias=b_sb[e][:])

        def rope_inplace(dst_chunks, cosR, sinS, n_total):
            for e in range(DC):
                sh = shp.tile([128, n_total], BF16, tag="shift", name="sh")
                for q in range(4):
                    src_q = (q // 2) * 2 + (1 - q % 2)  # 0<->32, 64<->96
                    nc.sync.dma_start(sh[ds(32 * q, 32), :],
                                      dst_chunks[e][ds(32 * src_q, 32), :])
                tmp = shp.tile([128, n_total], BF16, tag="ropetmp",
                               name="ropetmp")
                nc.vector.tensor_mul(tmp[:], sh[:], sinS[:])
                nc.vector.tensor_mul(dst_chunks[e][:], dst_chunks[e][:],
                                     cosR[:])
                nc.vector.tensor_add(dst_chunks[e][:], dst_chunks[e][:],
                                     tmp[:])

        proj_T(QT, wq_sb, bq_sb, xqT, SQ)
        rope_inplace(QT, cosR_q, sinS_q, SQ)
        proj_T(KT, wk_sb, bk_sb, xT, S)
        rope_inplace(KT, cosR_k, sinS_k, S)

        # V row-major with bias via ones-row rank-1 matmul
        for st in range(ST):
            for nt in range(2):
                p = pps.tile([128, 384], F32, tag="vproj", name="vproj_p")
                for dc in range(DC):
                    nc.tensor.matmul(p[:], xT[dc][:, ts(st, 128)],
                                     wv_sb[dc][:, ts(nt, 384)],
                                     start=(dc == 0), stop=False)
                nc.tensor.matmul(p[:], ones_row[:], bv_sb[:, ts(nt, 384)],
                                 start=False, stop=True)
                dst = Vaug[st].rearrange("p (h x) -> p h x", x=65)
                nc.scalar.copy(
                    dst[:, ds(nt * 6, 6), 0:64],
                    p.rearrange("p (h hd) -> p h hd", hd=64))
            va = Vaug[st].rearrange("p (h x) -> p h x", x=65)
            nc.gpsimd.memset(va[:, :, 64:65], 1.0)

    # ---- attention + o_proj ----
    wop = ctx.enter_context(tc.tile_pool(name="wop", bufs=1))
    with tc.tile_pool(name="wostage", bufs=2) as wo_stage:
        wo_sb = load_weight(woT, wo_stage, wop, "o")

    with (tc.tile_pool(name="scores_ps", bufs=2, space="PSUM") as sps,
          tc.tile_pool(name="pv_ps", bufs=2, space="PSUM") as pvps,
          tc.tile_pool(name="expp", bufs=6) as expp,
          tc.tile_pool(name="attn_sb", bufs=3) as asb):
        for hp in range(DC):          # head pair = e-chunk
            pv = [pvps.tile([65, SQ], F32, tag="pv", name=f"pv{i}")
                  for i in range(2)]
            for skt in range(ST):
                ex = []
                for i in range(2):  # head within pair
                    sc = sps.tile([128, SQ], F32, tag="sc", name="sc")
                    for j in range(SQ // 512):
                        nc.tensor.matmul(
                            sc[:, ts(j, 512)],
                            KT[hp][ds(64 * i, 64), ts(skt, 128)],
                            QT[hp][ds(64 * i, 64), ts(j, 512)],
                            start=True, stop=True,
                            tile_position=(64 * i, 0))
                    e = expp.tile([128, SQ], BF16, tag="exp", name="expt")
                    nc.scalar.activation(e[:], sc[:], AF.Exp, scale=0.125)
                    ex.append(e)
                for i in range(2):
                    h = 2 * hp + i
                    for j in range(SQ // 512):
                        nc.tensor.matmul(
                            pv[i][:, ts(j, 512)],
                            Vaug[skt][:, ds(h * 65, 65)],
                            ex[i][:, ts(j, 512)],
                            start=(skt == 0), stop=(skt == ST - 1))
            for i in range(2):
                # rowsum row (psum partition 64) -> sbuf, reshape to
                # partition-major via DMA, exact reciprocal on 128 lanes,
                # back to a row, broadcast via f32 rank-1 matmul
                rsrow = asb.tile([1, SQ], F32, tag="rsrow", name="rsrow")
                nc.vector.tensor_copy(rsrow[:], pv[i][ds(64, 1), :])
                c8 = asb.tile([128, SQ // 128], F32, tag="c8", name="c8")
                nc.sync.dma_start(c8[:], rsrow[:])
                r8 = asb.tile([128, SQ // 128], F32, tag="r8", name="r8")
                nc.vector.reciprocal(r8[:], c8[:])
                recb = asb.tile([1, SQ], F32, tag="recb", name="recb")
                nc.sync.dma_start(recb[:], r8[:])
                rb = sps.tile([64, SQ], F32, tag="sc", name="rb")
                for j in range(SQ // 512):
                    nc.tensor.matmul(rb[:, ts(j, 512)], ones64f[:],
                                     recb[:, ts(j, 512)],
                                     start=True, stop=True)
                rbs = asb.tile([64, SQ], BF16, tag="rbs", name="rbs")
                nc.vector.tensor_copy(rbs[:], rb[:])
                nc.vector.tensor_mul(attnT[hp][ds(64 * i, 64), :],
                                     pv[i][ds(0, 64), :], rbs[:])

    # ---- o_proj (row-major out) ----
    with (tc.tile_pool(name="o_ps", bufs=4, space="PSUM") as ops,
          tc.tile_pool(name="o_sb", bufs=4) as osb):
        for st in range(SQ // 128):
            for nt in range(2):
                p = ops.tile([128, 384], F32, tag="o", name="o_p")
                for dc in range(DC):
                    nc.tensor.matmul(p[:], attnT[dc][:, ts(st, 128)],
                                     wo_sb[dc][:, ts(nt, 384)],
                                     start=(dc == 0), stop=(dc == DC - 1))
                o = osb.tile([128, 384], F32, tag="o_out", name="o_out")
                nc.vector.tensor_copy(o[:], p[:])
                nc.sync.dma_start(out[ts(st, 128), ts(nt, 384)], o[:])


_NC_CACHE = None


def _get_nc():
    global _NC_CACHE
    if _NC_CACHE is None:
        _NC_CACHE = build_nc()
    return _NC_CACHE


def kernel(hidden_states, position_ids, wq, bq, wk, bk, wv, bv, wo,
           _trace=False):
    hidden_states = np.asarray(hidden_states, dtype=np.float32)
    position_ids = np.asarray(position_ids, dtype=np.int32)
    wqT = np.ascontiguousarray(np.asarray(wq, np.float32).T)
    wkT = np.ascontiguousarray(np.asarray(wk, np.float32).T)
    wvT = np.ascontiguousarray(np.asarray(wv, np.float32).T)
    woT = np.ascontiguousarray(np.asarray(wo, np.float32).T)
    bq_c = np.ascontiguousarray(np.asarray(bq, np.float32).reshape(D, 1))
    bk_c = np.ascontiguousarray(np.asarray(bk, np.float32).reshape(D, 1))
    bv_r = np.ascontiguousarray(np.asarray(bv, np.float32).reshape(1, D))

    nc = _get_nc()
    in_maps = []
    for core in range(N_CORES):
        b, half = core // 2, core % 2
        sl = slice(half * SQ, (half + 1) * SQ)
        in_maps.append({
            "hs": np.ascontiguousarray(hidden_states[b]),
            "hs_q": np.ascontiguousarray(hidden_states[b, sl]),
            "pos": np.ascontiguousarray(position_ids[b].reshape(1, S)),
            "pos_q": np.ascontiguousarray(position_ids[b, sl].reshape(1, SQ)),
            "wqT": wqT, "wkT": wkT, "wvT": wvT, "woT": woT,
            "bq": bq_c, "bk": bk_c, "bv": bv_r,
        })
    res = run_bass_kernel_spmd(nc, in_maps, list(range(N_CORES)),
                               trace=_trace)
    outp = np.empty((B, S, D), np.float32)
    for core in range(N_CORES):
        b, half = core // 2, core % 2
        outp[b, half * SQ:(half + 1) * SQ] = res.results[core]["out"]
    if _trace:
        kernel._last_exec_time_ns = res.exec_time_ns
        kernel._last_results = res
    return outp


# revision 13
# speedup vs baseline: 1.2310x; 1.0700x over previous
"""Multi-head attention (B=4, S=2048, D=768, H=12) on 8 TRN2 NeuronCores.

Sharding: core = (batch b, query-half). Each core computes Q for its 1024
query rows and full-sequence K/V for its batch (K/V projection duplicated
across the 2 cores sharing a batch -> zero collectives), then SDPA + o_proj
for its rows. Output rows are disjoint across cores.

Device layout: "T-layout" [feature, seq] with features on partitions.
 - Q/K projected as QT/KT [768, S*] (bias fused via per-partition scalar add)
 - RoPE applied in T-layout (partition-shifted copy via DMA, sin/cos tables
   built on device from position_ids via rank-1 matmul + mod-2pi + Sin LUT)
 - scores computed TRANSPOSED: psum[sk, sq] = KT_h.T @ QT_h (two heads packed
   per matmul pair via tile_position row tiling, K=64 each)
 - exp fused into the psum->sbuf eviction on ScalarE (scale=1/8, no max-sub:
   scores are ~N(0,1) so exp overflow is impossible)
 - P@V directly consumes exp(scoresT) as the moving operand; V kept row-major
   [S, 768] with a ones column appended per head -> psum row 64 = softmax
   denominator for free
 - normalization deferred: attnT tiles scaled by broadcast 1/rowsum during
   psum eviction; o_proj emits row-major [sq, 768] f32.
"""

from contextlib import ExitStack

import numpy as np

import concourse.bass as bass
import concourse.bacc as bacc
import concourse.mybir as mybir
import concourse.tile as tile
from concourse.bass import ds, ts
from concourse.bass_utils import run_bass_kernel_spmd
from concourse.masks import make_identity

F32 = mybir.dt.float32
BF16 = mybir.dt.bfloat16
I32 = mybir.dt.int32
AF = mybir.ActivationFunctionType

B, S, D, H = 4, 2048, 768, 12
HD = 64
SQ = 1024          # query rows per core
DC = D // 128      # 6 d-chunks
ST = S // 128      # 16 seq tiles of 128
ROPE_BASE = 10000.0
TWO_PI = float(2.0 * np.pi)
N_CORES = 8


def build_nc():
    nc = bacc.Bacc("TRN2", target_bir_lowering=False, debug=False,
                   num_devices=N_CORES)

    hs = nc.dram_tensor("hs", [S, D], F32, kind="ExternalInput")
    hs_q = nc.dram_tensor("hs_q", [SQ, D], F32, kind="ExternalInput")
    pos = nc.dram_tensor("pos", [1, S], I32, kind="ExternalInput")
    pos_q = nc.dram_tensor("pos_q", [1, SQ], I32, kind="ExternalInput")
    wqT = nc.dram_tensor("wqT", [D, D], F32, kind="ExternalInput")
    wkT = nc.dram_tensor("wkT", [D, D], F32, kind="ExternalInput")
    wvT = nc.dram_tensor("wvT", [D, D], F32, kind="ExternalInput")
    woT = nc.dram_tensor("woT", [D, D], F32, kind="ExternalInput")
    bq = nc.dram_tensor("bq", [D, 1], F32, kind="ExternalInput")
    bk = nc.dram_tensor("bk", [D, 1], F32, kind="ExternalInput")
    bv = nc.dram_tensor("bv", [1, D], F32, kind="ExternalInput")
    out = nc.dram_tensor("out", [SQ, D], F32, kind="ExternalOutput")

    # turns-per-position for each of the 32 rope frequencies (constant)
    invf_turns_np = ((1.0 / ROPE_BASE) ** (np.arange(32) / 32.0) / TWO_PI
                     ).reshape(1, 32).astype(np.float32)
    invf_dram = nc.inline_tensor(invf_turns_np, name="invf_turns")

    with tile.TileContext(nc) as tc:
        _body(nc, tc, hs, hs_q, pos, pos_q, wqT, wkT, wvT, woT,
              bq, bk, bv, out, invf_dram)
    nc.compile()
    return nc


def _body(nc, tc, hs, hs_q, pos, pos_q, wqT, wkT, wvT, woT,
          bq, bk, bv, out, invf_dram):
  with ExitStack() as ctx:
    const = ctx.enter_context(tc.tile_pool(name="const", bufs=1))
    persist = ctx.enter_context(tc.tile_pool(name="persist", bufs=1))

    # ---- constants ----
    ident = const.tile([128, 128], F32, tag="ident")
    make_identity(nc, ident[:])
    ones_row = const.tile([1, 128], BF16, tag="ones_row")
    nc.gpsimd.memset(ones_row[:], 1.0)
    ones64f = const.tile([1, 64], F32, tag="ones64f")
    nc.gpsimd.memset(ones64f[:], 1.0)
    invf = const.tile([1, 32], F32, tag="invf")
    nc.sync.dma_start(invf[:], invf_dram[:])

    # ---- rope tables (device-side, from position ids) ----
    def rope_tables(pos_dram, n, name):
        with (tc.tile_pool(name=f"rope_{name}", bufs=1) as rp,
              tc.tile_pool(name=f"rope_ps_{name}", bufs=1,
                           space="PSUM") as rps):
            posi = rp.tile([1, n], I32, tag="posi")
            nc.sync.dma_start(posi[:], pos_dram[:])
            posf = rp.tile([1, n], F32, tag="posf")
            nc.vector.tensor_copy(posf[:], posi[:])
            turns = rps.tile([32, n], F32, tag="turns")
            for i in range(n // 512):
                nc.tensor.matmul(turns[:, ts(i, 512)], invf[:],
                                 posf[:, ts(i, 512)], start=True, stop=True)
            sincos = {}
            for which, shift in (("sin", 0.0), ("cos", 0.25)):
                tsh = rp.tile([32, n], F32, tag="tsh")
                nc.vector.tensor_scalar_add(tsh[:], turns[:], shift)
                ti = rp.tile([32, n], I32, tag="ti")
                nc.vector.tensor_copy(ti[:], tsh[:])
                tif = rp.tile([32, n], F32, tag="tif")
                nc.vector.tensor_copy(tif[:], ti[:])
                frac = rp.tile([32, n], F32, tag="frac")
                nc.vector.tensor_sub(frac[:], tsh[:], tif[:])
                val = rp.tile([32, n], BF16, tag=f"val_{which}")
                nc.scalar.activation(val[:], frac[:], AF.Sin, scale=TWO_PI)
                sincos[which] = val
            sin32, cos32 = sincos["sin"], sincos["cos"]
            nsin32 = rp.tile([32, n], BF16, tag="nsin")
            nc.vector.tensor_scalar_mul(nsin32[:], sin32[:], -1.0)
            cosR = persist.tile([128, n], BF16, tag=f"cosR_{name}")
            sinS = persist.tile([128, n], BF16, tag=f"sinS_{name}")
            for q in range(4):
                nc.gpsimd.dma_start(cosR[ds(32 * q, 32), :], cos32[:])
                src = nsin32 if q % 2 == 0 else sin32
                nc.gpsimd.dma_start(sinS[ds(32 * q, 32), :], src[:])
        return cosR, sinS

    cosR_k, sinS_k = rope_tables(pos, S, "k")
    cosR_q, sinS_q = rope_tables(pos_q, SQ, "q")

    # persistent activation tensors
    QT = [persist.tile([128, SQ], BF16, tag=f"QT{e}", name=f"QT{e}")
          for e in range(DC)]
    KT = [persist.tile([128, S], BF16, tag=f"KT{e}", name=f"KT{e}")
          for e in range(DC)]
    Vaug = [persist.tile([128, H * 65], BF16, tag=f"Vaug{st}",
                         name=f"Vaug{st}") for st in range(ST)]
    attnT = [persist.tile([128, SQ], BF16, tag=f"attnT{e}", name=f"attnT{e}")
             for e in range(DC)]

    # load + cast one [768,768] weight into 6 bf16 chunks
    def load_weight(wT_dram, stage_pool, dst_pool, name):
        chunks = []
        for dc in range(DC):
            f = stage_pool.tile([128, D], F32, tag="wld", name="wld")
            nc.sync.dma_start(f[:], wT_dram[ts(dc, 128), :])
            c = dst_pool.tile([128, D], BF16, tag=f"w_{name}{dc}",
                              name=f"w_{name}{dc}")
            nc.vector.tensor_copy(c[:], f[:])
            chunks.append(c)
        return chunks

    # ---- projection super-stage ----
    with (tc.tile_pool(name="xt", bufs=1) as xt_pool,
          tc.tile_pool(name="hsload", bufs=5) as hl,
          tc.tile_pool(name="qkv_w", bufs=1) as qkv_w,
          tc.tile_pool(name="wstage", bufs=3) as wstage,
          tc.tile_pool(name="shift", bufs=2) as shp,
          tc.tile_pool(name="xpose_ps", bufs=2, space="PSUM") as xps,
          tc.tile_pool(name="proj_ps", bufs=3, space="PSUM") as pps):

        # transpose hidden states -> xT [768, S], xqT [768, SQ] (bf16)
        xT = [xt_pool.tile([128, S], BF16, tag=f"xT{dc}", name=f"xT{dc}")
              for dc in range(DC)]
        xqT = [xt_pool.tile([128, SQ], BF16, tag=f"xqT{dc}", name=f"xqT{dc}")
               for dc in range(DC)]

        def transpose_in(dst, src_dram, n_total):
            for sg in range(n_total // 512):
                tl = []
                for j in range(4):
                    t = hl.tile([128, D], F32, tag="hs_t", name="hs_t")
                    nc.sync.dma_start(t[:], src_dram[ts(sg * 4 + j, 128), :])
                    tl.append(t)
                for dc in range(DC):
                    p = xps.tile([128, 512], F32, tag="xp", name="xp")
                    for j in range(4):
                        nc.tensor.transpose(p[:, ts(j, 128)],
                                            tl[j][:, ts(dc, 128)], ident[:])
                    nc.vector.tensor_copy(dst[dc][:, ts(sg, 512)], p[:])

        transpose_in(xT, hs, S)
        transpose_in(xqT, hs_q, SQ)

        wq_sb = load_weight(wqT, wstage, qkv_w, "q")
        wk_sb = load_weight(wkT, wstage, qkv_w, "k")
        wv_sb = load_weight(wvT, wstage, qkv_w, "v")

        bq_sb = [qkv_w.tile([128, 1], F32, tag=f"bq{e}", name=f"bq{e}")
                 for e in range(DC)]
        bk_sb = [qkv_w.tile([128, 1], F32, tag=f"bk{e}", name=f"bk{e}")
                 for e in range(DC)]
        for e in range(DC):
            nc.sync.dma_start(bq_sb[e][:], bq[ts(e, 128), :])
            nc.sync.dma_start(bk_sb[e][:], bk[ts(e, 128), :])
        bv_f = qkv_w.tile([1, D], F32, tag="bv_f", name="bv_f")
        nc.sync.dma_start(bv_f[:], bv[:])
        bv_sb = qkv_w.tile([1, D], BF16, tag="bv", name="bv_sb")
        nc.vector.tensor_copy(bv_sb[:], bv_f[:])

        def proj_T(dst, w_sb, b_sb, x_chunks, n_total):
            for e in range(DC):
                for i in range(n_total // 512):
                    p = pps.tile([128, 512], F32, tag="proj", name="proj_p")
                    for dc in range(DC):
                        nc.tensor.matmul(p[:], w_sb[dc][:, ts(e, 128)],
                                         x_chunks[dc][:, ts(i, 512)],
                                         start=(dc == 0), stop=(dc == DC - 1))
                    nc.vector.tensor_scalar_add(dst[e][:, ts(i, 512)], p[:],
                                                b_sb[e][:])

        def rope_inplace(dst_chunks, cosR, sinS, n_total):
            for e in range(DC):
                sh = shp.tile([128, n_total], BF16, tag="shift", name="sh")
                for q in range(4):
                    src_q = (q // 2) * 2 + (1 - q % 2)  # 0<->32, 64<->96
                    nc.gpsimd.dma_start(sh[ds(32 * q, 32), :],
                                        dst_chunks[e][ds(32 * src_q, 32), :])
                tmp = shp.tile([128, n_total], BF16, tag="ropetmp",
                               name="ropetmp")
                nc.vector.tensor_mul(tmp[:], sh[:], sinS[:])
                nc.vector.tensor_mul(dst_chunks[e][:], dst_chunks[e][:],
                                     cosR[:])
                nc.vector.tensor_add(dst_chunks[e][:], dst_chunks[e][:],
                                     tmp[:])

        proj_T(QT, wq_sb, bq_sb, xqT, SQ)
        rope_inplace(QT, cosR_q, sinS_q, SQ)
        proj_T(KT, wk_sb, bk_sb, xT, S)
        rope_inplace(KT, cosR_k, sinS_k, S)

        # V row-major with bias via ones-row rank-1 matmul
        for st in range(ST):
            for nt in range(2):
                p = pps.tile([128, 384], F32, tag="vproj", name="vproj_p")
                for dc in range(DC):
                    nc.tensor.matmul(p[:], xT[dc][:, ts(st, 128)],
                                     wv_sb[dc][:, ts(nt, 384)],
                                     start=(dc == 0), stop=False)
                nc.tensor.matmul(p[:], ones_row[:], bv_sb[:, ts(nt, 384)],
                                 start=False, stop=True)
                dst = Vaug[st].rearrange("p (h x) -> p h x", x=65)
                nc.vector.tensor_copy(
                    dst[:, ds(nt * 6, 6), 0:64],
                    p.rearrange("p (h hd) -> p h hd", hd=64))
            va = Vaug[st].rearrange("p (h x) -> p h x", x=65)
            nc.gpsimd.memset(va[:, :, 64:65], 1.0)

    # ---- attention + o_proj ----
    wop = ctx.enter_context(tc.tile_pool(name="wop", bufs=1))
    with tc.tile_pool(name="wostage", bufs=2) as wo_stage:
        wo_sb = load_weight(woT, wo_stage, wop, "o")

    with (tc.tile_pool(name="scores_ps", bufs=3, space="PSUM") as sps,
          tc.tile_pool(name="pv_ps", bufs=1, space="PSUM") as pvps,
          tc.tile_pool(name="expp", bufs=34) as expp,
          tc.tile_pool(name="attn_sb", bufs=3) as asb):
        for hp in range(DC):          # head pair = e-chunk
            # all score matmuls for both heads first: PE runs ahead and
            # ScalarE streams exps continuously behind it; exp tiles are
            # retained in SBUF, then PV runs per head
            ex = [[None] * ST, [None] * ST]
            for skt in range(ST):
                for i in range(2):  # head within pair
                    sc = sps.tile([128, SQ], F32, tag="sc", name="sc")
                    for j in range(SQ // 512):
                        nc.tensor.matmul(
                            sc[:, ts(j, 512)],
                            KT[hp][ds(64 * i, 64), ts(skt, 128)],
                            QT[hp][ds(64 * i, 64), ts(j, 512)],
                            start=True, stop=True,
                            tile_position=(64 * i, 0))
                    e = expp.tile([128, SQ], BF16, tag="exp", name="expt")
                    nc.scalar.activation(e[:], sc[:], AF.Exp, scale=0.125)
                    ex[i][skt] = e
            pv = [None, None]
            for i in range(2):
                h = 2 * hp + i
                pv[i] = pvps.tile([65, SQ], F32, tag="pv", name=f"pv{i}")
                for skt in range(ST):
                    for j in range(SQ // 512):
                        nc.tensor.matmul(
                            pv[i][:, ts(j, 512)],
                            Vaug[skt][:, ds(h * 65, 65)],
                            ex[i][skt][:, ts(j, 512)],
                            start=(skt == 0), stop=(skt == ST - 1))
            for i in range(2):
                # rowsum row (psum partition 64) -> sbuf, reshape to
                # partition-major via DMA, exact reciprocal on 128 lanes,
                # back to a row, broadcast via f32 rank-1 matmul
                rsrow = asb.tile([1, SQ], F32, tag="rsrow", name="rsrow")
                nc.vector.tensor_copy(rsrow[:], pv[i][ds(64, 1), :])
                c8 = asb.tile([128, SQ // 128], F32, tag="c8", name="c8")
                nc.gpsimd.dma_start(c8[:], rsrow[:])
                r8 = asb.tile([128, SQ // 128], F32, tag="r8", name="r8")
                nc.vector.reciprocal(r8[:], c8[:])
                recb = asb.tile([1, SQ], F32, tag="recb", name="recb")
                nc.gpsimd.dma_start(recb[:], r8[:])
                rb = sps.tile([64, SQ], F32, tag="sc", name="rb")
                for j in range(SQ // 512):
                    nc.tensor.matmul(rb[:, ts(j, 512)], ones64f[:],
                                     recb[:, ts(j, 512)],
                                     start=True, stop=True)
                rbs = asb.tile([64, SQ], BF16, tag="rbs", name="rbs")
                nc.vector.tensor_copy(rbs[:], rb[:])
                nc.vector.tensor_mul(attnT[hp][ds(64 * i, 64), :],
                                     pv[i][ds(0, 64), :], rbs[:])

    # ---- o_proj (row-major out) ----
    with (tc.tile_pool(name="o_ps", bufs=4, space="PSUM") as ops,
          tc.tile_pool(name="o_sb", bufs=4) as osb):
        for st in range(SQ // 128):
            for nt in range(2):
                p = ops.tile([128, 384], F32, tag="o", name="o_p")
                for dc in range(DC):
                    nc.tensor.matmul(p[:], attnT[dc][:, ts(st, 128)],
                                     wo_sb[dc][:, ts(nt, 384)],
                                     start=(dc == 0), stop=(dc == DC - 1))
                o = osb.tile([128, 384], F32, tag="o_out", name="o_out")
                nc.vector.tensor_copy(o[:], p[:])
                nc.sync.dma_start(out[ts(st, 128), ts(nt, 384)], o[:])


_NC_CACHE = None


def _get_nc():
    global _NC_CACHE
    if _NC_CACHE is None:
        _NC_CACHE = build_nc()
    return _NC_CACHE


def kernel(hidden_states, position_ids, wq, bq, wk, bk, wv, bv, wo,
           _trace=False):
    hidden_states = np.asarray(hidden_states, dtype=np.float32)
    position_ids = np.asarray(position_ids, dtype=np.int32)
    wqT = np.ascontiguousarray(np.asarray(wq, np.float32).T)
    wkT = np.ascontiguousarray(np.asarray(wk, np.float32).T)
    wvT = np.ascontiguousarray(np.asarray(wv, np.float32).T)
    woT = np.ascontiguousarray(np.asarray(wo, np.float32).T)
    bq_c = np.ascontiguousarray(np.asarray(bq, np.float32).reshape(D, 1))
    bk_c = np.ascontiguousarray(np.asarray(bk, np.float32).reshape(D, 1))
    bv_r = np.ascontiguousarray(np.asarray(bv, np.float32).reshape(1, D))

    nc = _get_nc()
    in_maps = []
    for core in range(N_CORES):
        b, half = core // 2, core % 2
        sl = slice(half * SQ, (half + 1) * SQ)
        in_maps.append({
            "hs": np.ascontiguousarray(hidden_states[b]),
            "hs_q": np.ascontiguousarray(hidden_states[b, sl]),
            "pos": np.ascontiguousarray(position_ids[b].reshape(1, S)),
            "pos_q": np.ascontiguousarray(position_ids[b, sl].reshape(1, SQ)),
            "wqT": wqT, "wkT": wkT, "wvT": wvT, "woT": woT,
            "bq": bq_c, "bk": bk_c, "bv": bv_r,
        })
    res = run_bass_kernel_spmd(nc, in_maps, list(range(N_CORES)),
                               trace=_trace)
    outp = np.empty((B, S, D), np.float32)
    for core in range(N_CORES):
        b, half = core // 2, core % 2
        outp[b, half * SQ:(half + 1) * SQ] = res.results[core]["out"]
    if _trace:
        kernel._last_exec_time_ns = res.exec_time_ns
        kernel._last_results = res
    return outp
